# revision 4
# baseline (speedup 1.0000x reference)
"""Depth2Normals Trainium2 Bass kernel v2 (8 NeuronCores, one image per core).

Pipeline per image: 9x9 joint bilateral filter (zeros invalid, LNEG sentinel
trick), K rounds of 3x3 median hole-fill (K determined host-side), then
back-projection + central-difference cross-product normals.

v2 bilateral restructure (vs the all-DVE baseline):
  filt = x - S/den  with  S = sum_k w_k*diff_k,  den = 1 + sum_k w_k,
  diff_k = x - nb_k,  w_k = ws_k*exp(-c*diff_k^2).
Per tap the DVE does only 2 tensor-tensor ops (diff, wd=w*diff); the ACT
computes w in ONE op via Derivative_Erf ((2/sqrt(pi))*exp(-x^2), table clamps
to 0 at large args); the per-tap spatial scale ws_k*sqrt(pi)/2 is folded into
PE scaled-identity fp32r matmuls that accumulate S and den in PSUM (exact f32
accumulate; fp32r operand rounding is ~2.4e-4 rel, measured end-to-end error
~5e-4 vs the f32 reference).  PSUM capacity forces 3 passes over the free dim
(1024/1024/382).

Median fill: full Batcher sort-8 (19 comparators) of the neighbor views with
0-as-invalid sentinel; count thresholds come free from order-statistic
positivity ([cnt>=k] == [s_{8-k}>0]) so no separate count chain.

Layout: 480x640 image row-padded to 648 cols, flattened to 311040 elements,
128 partitions x 2430; stencil inputs live in per-partition halo buffers
built by overlapped-window DMA from padded DRAM staging (all taps are pure
free-dim offsets).

build_program(K, R) emits the pipeline R times back-to-back (R-slope HW
timing cancels the axon tunnel dispatch overhead).
"""

import math
import os
import numpy as np

import concourse.bass as bass
import concourse.mybir as mybir

F32 = mybir.dt.float32
F32R = mybir.dt.float32r
OP = mybir.AluOpType
AF = mybir.ActivationFunctionType

# ---------------- geometry ----------------
B, H, W = 8, 480, 640
WP = W + 8                 # padded row width
NPIX = H * WP              # 311040
NPART = 128
CH = NPIX // NPART         # 2430
R4 = 4
HALO4 = R4 * WP + R4       # 2596
XHW = CH + 2 * HALO4       # 7622
G4 = 2600
HALO1 = WP + 1             # 649
MFW = CH + 2 * HALO1       # 3728
G1 = 652
PSZ = G4 + NPIX + G4       # 316240
QSZ = G1 + NPIX + G1       # 312344

INV2SR = 50.0
SQC = float(np.float32(math.sqrt(INV2SR)))   # ACT scale for Derivative_Erf
LNEG = -30.0

# bilateral passes over the free dim; PSUM: S and den per pass (<=1024 f32)
PASSES = [(0, 1024), (1024, 1024), (2048, 382)]
CHUNKS = {1024: [(0, 512), (512, 512)], 382: [(0, 382)]}

# taps sorted by r^2 so equal-weight matmuls are adjacent
TAPS = sorted(((dy, dx) for dy in range(-R4, R4 + 1)
               for dx in range(-R4, R4 + 1) if (dy, dx) != (0, 0)),
              key=lambda t: (t[0] * t[0] + t[1] * t[1]))
R2S = sorted({dy * dy + dx * dx for dy, dx in TAPS})   # 14 distinct values
R2COL = {r2: j for j, r2 in enumerate(R2S)}
NID = len(R2S)

ARENA_COLS = 42000
DRAIN_VS = os.environ.get("KDRAIN", "0") == "1"


class Buf:
    def __init__(self, arena, name, start, n):
        self.arena, self.name, self.start, self.n = arena, name, start, n

    def v(self, a=0, n=None):
        if n is None:
            n = self.n - a
        return self.arena.t[:, self.start + a: self.start + a + n]

    def free(self):
        self.arena.free(self)


class Arena:
    def __init__(self, tensor, cols):
        self.t = tensor
        self.free_list = [(0, cols)]
        self.live = {}

    def alloc(self, name, n):
        na = (n + 15) & ~15
        for i, (s, ln) in enumerate(self.free_list):
            if ln >= na:
                self.live[name] = (s, na)
                if ln == na:
                    self.free_list.pop(i)
                else:
                    self.free_list[i] = (s + na, ln - na)
                return Buf(self, name, s, n)
        raise MemoryError(f"arena OOM for {name}:{n} free={self.free_list}")

    def free(self, buf):
        s, n = self.live.pop(buf.name)
        self.free_list.append((s, n))
        self.free_list.sort()
        out = []
        for seg in self.free_list:
            if out and out[-1][0] + out[-1][1] == seg[0]:
                out[-1] = (out[-1][0], out[-1][1] + seg[1])
            else:
                out.append(seg)
        self.free_list = out


class Prog:
    """Planned per-engine instruction streams with semaphore accounting.

    'v' = DVE, 's' = ACT, 'y' = SP (sync/HWDGE dma), 't' = PE.
    Compute engines execute in order: one semaphore each, +1 per instruction,
    waits are (engine, value).  Each DMA slot has its own semaphore (+16 per
    completion); token ('D', name, n) waits for sem >= 16n.
    """

    def __init__(self):
        self.items = {k: [] for k in 'vsyt'}
        self.cnt = {k: 0 for k in 'vsyt'}
        self.last_wait = {}
        self.dma_count = {}

    def op(self, eng, fn, inc=1, waits=()):
        real = []
        for wt in waits:
            if wt is None:
                continue
            if wt[0] == 'D':
                _, name, cnt = wt
                k = (eng, 'D', name)
                if self.last_wait.get(k, -1) >= cnt:
                    continue
                self.last_wait[k] = cnt
                real.append(wt)
                continue
            se, val = wt
            if val is None or val <= 0:
                continue
            if se == eng:
                continue
            k = (eng, se)
            if self.last_wait.get(k, -1) >= val:
                continue
            self.last_wait[k] = val
            real.append((se, val))
        self.items[eng].append((real, fn, inc))
        if not isinstance(inc, tuple):
            self.cnt[eng] += inc
        return self.cnt[eng]

    def dma(self, name, fn, waits=()):
        cnt = self.dma_count.get(name, 0) + 1
        self.dma_count[name] = cnt
        self.op('y', fn, inc=('D', name, cnt), waits=waits)
        return ('D', name, cnt)


def build_program(K, R=1):
    nc = bass.Bass("TRN2", target_bir_lowering=False, debug=False)
    AP = bass.AP

    depth = nc.dram_tensor("depth", [H, W], F32, kind="ExternalInput")
    intr8 = nc.dram_tensor("intr8", [NPART, 8], F32, kind="ExternalInput")
    ucol = nc.dram_tensor("ucol", [QSZ], F32, kind="ExternalInput")
    vrow = nc.dram_tensor("vrow", [QSZ], F32, kind="ExternalInput")
    ident = nc.dram_tensor("ident", [NPART, NID * NPART], F32,
                           kind="ExternalInput")
    out = nc.dram_tensor("out", [3, H, W], F32, kind="ExternalOutput")

    P = nc.dram_tensor("Pstage", [PSZ], F32)
    Q = nc.dram_tensor("Qstage", [QSZ], F32)
    O = nc.dram_tensor("Ostage", [3, NPIX], F32)

    arena_t = nc.alloc_sbuf_tensor("arena", [NPART, ARENA_COLS], F32)
    ar = Arena(arena_t, ARENA_COLS)
    smalls = nc.alloc_sbuf_tensor("smalls", [NPART, 16], F32)
    idr = nc.alloc_sbuf_tensor("idr", [NPART, NID * NPART], F32)
    # fp32r-consumed buffers (w~, wd~) need dedicated, never-recycled memory:
    # the BIR verifier requires every writer overlapping an fp32r matmul input
    # region to be a rounding producer.
    wbuf = nc.alloc_sbuf_tensor("wbuf", [NPART, 4096], F32)
    mask8 = nc.alloc_sbuf_tensor("mask8", [NPART, 4 * CH], mybir.dt.int8)
    psS = nc.alloc_psum_tensor("psS", [NPART, 1024], F32)
    psD = nc.alloc_psum_tensor("psD", [NPART, 1024], F32)

    pr = Prog()

    ap_cx, ap_cy = smalls[:, 2:3], smalls[:, 3:4]
    ap_rfx, ap_rfy = smalls[:, 8:9], smalls[:, 9:10]
    ap_flag = smalls[:, 4:5]

    def V(fn, waits=()):
        return pr.op('v', fn, waits=waits)

    def TT(o, a, b, op, waits=()):
        return V(lambda e, o=o, a=a, b=b, op=op: e.tensor_tensor(o, a, b, op),
                 waits=waits)

    def idv(r2):
        c = R2COL[r2] * NPART
        return idr[:, c:c + NPART].bitcast(F32R)

    # ---- one-time: ident upload + fp32r rounding copy (iteration 0 only) ----
    idtmp = ar.alloc("idtmp", NID * NPART)
    d_id = pr.dma("id", lambda e: e.dma_start(out=idtmp.v(), in_=ident[:, :]))
    v_id = V(lambda e: e.tensor_copy(idr[:, :].bitcast(F32R), idtmp.v()),
             waits=[d_id])
    idtmp.free()

    state = {'prev_tq': v_id}

    def emit_iter(prev_fouts):
        # ================= phase 0: staging prep =================
        zt = ar.alloc("zt", 3008)
        v_zt = pr.op('v', lambda e: e.memset(zt.v(), 0.0), waits=prev_fouts)

        pz_a, pz_b = 2470, PSZ - 2470 * NPART           # 316160 + 80
        d_z1 = pr.dma("z1", lambda e: e.dma_start(
            out=AP(P, 0, [[pz_a, NPART], [1, pz_a]]), in_=zt.v(0, pz_a)),
            waits=[('v', v_zt)])
        d_z2 = pr.dma("z2", lambda e: e.dma_start(
            out=AP(P, pz_a * NPART, [[pz_b, 1], [1, pz_b]]),
            in_=arena_t[0:1, zt.start: zt.start + pz_b]),
            waits=[('v', v_zt)])
        qz_a, qz_b = 2440, QSZ - 2440 * NPART           # 312320 + 24
        d_z3 = pr.dma("z3", lambda e: e.dma_start(
            out=AP(Q, 0, [[qz_a, NPART], [1, qz_a]]), in_=zt.v(0, qz_a)),
            waits=[('v', v_zt)])
        d_z4 = pr.dma("z4", lambda e: e.dma_start(
            out=AP(Q, qz_a * NPART, [[qz_b, 1], [1, qz_b]]),
            in_=arena_t[0:1, zt.start: zt.start + qz_b]),
            waits=[('v', v_zt)])
        d_depth = pr.dma("dep", lambda e: e.dma_start(
            out=AP(P, G4 + 4, [[WP, H], [1, W]]),
            in_=AP(depth, 0, [[W, H], [1, W]])),
            waits=[d_z2])
        xh = ar.alloc("xh", XHW)
        d_xh = pr.dma("xh", lambda e: e.dma_start(
            out=xh.v(), in_=AP(P, G4 - HALO4, [[CH, NPART], [1, XHW]])),
            waits=[d_depth])

        d_intr = pr.dma("intr", lambda e: e.dma_start(
            out=smalls[:, 0:8], in_=intr8[:, 0:8]), waits=[('v', v_zt)])
        V(lambda e: e.reciprocal(smalls[:, 8:10], smalls[:, 0:2]), waits=[d_intr])

        # pad-column mask (1 at the 640 real columns, 0 at the 8 pad columns)
        pm = ar.alloc("pm", CH)
        tpm = ar.alloc("tpm", CH)
        d_pm = pr.dma("pm", lambda e: e.dma_start(
            out=pm.v(), in_=AP(ucol, G1, [[CH, NPART], [1, CH]])),
            waits=[('v', v_zt)])
        V(lambda e: e.tensor_scalar(tpm.v(), pm.v(), 639.5, None, OP.is_le),
          waits=[d_pm])
        V(lambda e: e.scalar_tensor_tensor(
            pm.v(), pm.v(), -0.5, tpm.v(), OP.is_ge, OP.mult))
        tpm.free()

        d_all0 = [d_z1, d_z2, d_z3, d_z4, d_depth, d_xh, d_intr, d_pm]
        zt.free()

        # ============ phase 1: x' = (d>0 ? d : LNEG), in place on xh ========
        tmp_xh = ar.alloc("tmp_xh", XHW)
        V(lambda e: e.tensor_scalar(
            tmp_xh.v(), xh.v(), 0.0, LNEG, OP.is_le, OP.mult), waits=d_all0)
        V(lambda e: e.scalar_tensor_tensor(
            xh.v(), tmp_xh.v(), 0.0, xh.v(), OP.bypass, OP.add))
        tmp_xh.free()

        OFF4 = HALO4
        xc_full = xh.v(OFF4, CH)
        mp = ar.alloc("mp", CH)
        v_mp = V(lambda e: e.tensor_scalar(mp.v(), xc_full, 0.0, None, OP.is_gt))

        # ================= phase 2: bilateral (3 passes) =================
        ff = ar.alloc("ff", CH)       # filt output, full width
        diff = [ar.alloc("diff0", 1024), ar.alloc("diff1", 1024)]
        sden = ar.alloc("sden", 1024)
        rden = ar.alloc("rden", 1024)
        tqb = ar.alloc("tq", 1024)

        class FixedBuf:
            def __init__(self, start):
                self.start = start

            def v(self, a=0, n=1024):
                return wbuf[:, self.start + a: self.start + a + n]

            def free(self):
                pass

        wb = [FixedBuf(0), FixedBuf(1024)]
        wd = [FixedBuf(2048), FixedBuf(3072)]

        NT = len(TAPS)
        for (po, pw) in PASSES:
            chunks = CHUNKS[pw]
            xc = xh.v(OFF4 + po, pw)

            s_w = [None] * NT            # ACT count after w~ of tap k
            v_diff = [None] * NT         # DVE count after diff of tap k
            v_wd = [None] * NT           # DVE count after wd of tap k
            t_den = [None] * NT          # PE count after den matmuls of tap k
            t_S = [None] * NT            # PE count after S matmuls of tap k

            for k, (dy, dx) in enumerate(TAPS):
                bi = k % 2
                nb = xh.v(OFF4 + po + WP * dy + dx, pw)
                # diff = x' - nb'   (WAR: ACT of tap k-2 reads diff[bi])
                w_ = [('s', s_w[k - 2])] if k >= 2 else list(d_all0)
                v_diff[k] = V(lambda e, bi=bi, nb=nb, pw=pw, xc=xc:
                              e.tensor_tensor(diff[bi].v(0, pw), xc, nb,
                                              OP.subtract), waits=w_)
                # w~ = (2/sqrt(pi))*exp(-c*diff^2), fp32r-rounded output
                # (WAR: PE den matmuls of tap k-2 read wb[bi])
                w_ = [('v', v_diff[k])]
                if k >= 2:
                    w_.append(('t', t_den[k - 2]))
                s_w[k] = pr.op('s', lambda e, bi=bi, pw=pw: e.activation(
                    wb[bi].v(0, pw).bitcast(F32R), diff[bi].v(0, pw),
                    AF.Derivative_Erf, scale=SQC), waits=w_)
                # wd~ = w~ * diff, fp32r-rounded (WAR: PE S matmuls k-2)
                w_ = [('s', s_w[k])]
                if k >= 2:
                    w_.append(('t', t_S[k - 2]))
                v_wd[k] = V(lambda e, bi=bi, pw=pw: e.tensor_tensor(
                    wd[bi].v(0, pw).bitcast(F32R), wb[bi].v(0, pw),
                    diff[bi].v(0, pw), OP.mult), waits=w_)

                # PE: den += s_k * w~ ; S += s_k * wd~
                idap = idv(dy * dy + dx * dx)
                w_ = [('s', s_w[k])]
                if k == 0:
                    w_.append(('v', state['prev_tq']))
                for (c0, cw) in chunks:
                    pr.op('t', lambda e, c0=c0, cw=cw, bi=bi, idap=idap, k=k:
                          e.matmul(psD[:, c0:c0 + cw], idap,
                                   wb[bi].v(c0, cw).bitcast(F32R),
                                   start=(k == 0), stop=(k == NT - 1),
                                   skip_group_check=True), waits=w_)
                    w_ = []
                t_den[k] = pr.cnt['t']
                w_ = [('v', v_wd[k])]
                for (c0, cw) in chunks:
                    pr.op('t', lambda e, c0=c0, cw=cw, bi=bi, idap=idap, k=k:
                          e.matmul(psS[:, c0:c0 + cw], idap,
                                   wd[bi].v(c0, cw).bitcast(F32R),
                                   start=(k == 0), stop=(k == NT - 1),
                                   skip_group_check=True), waits=w_)
                    w_ = []
                t_S[k] = pr.cnt['t']

            # ---- pass finalize: ff[po:po+pw] = (x - S/(den+1)) * (x>0) ----
            V(lambda e, pw=pw: e.tensor_scalar(
                sden.v(0, pw), psD[:, 0:pw], 1.0, None, OP.add),
              waits=[('t', t_den[NT - 1])])
            V(lambda e, pw=pw: e.reciprocal(rden.v(0, pw), sden.v(0, pw)))
            V(lambda e, pw=pw: e.tensor_tensor(
                tqb.v(0, pw), psS[:, 0:pw], rden.v(0, pw), OP.mult),
              waits=[('t', t_S[NT - 1])])
            state['prev_tq'] = pr.cnt['v']
            V(lambda e, pw=pw, po=po, xc=xc: e.scalar_tensor_tensor(
                ff.v(po, pw), tqb.v(0, pw), -1.0, xc, OP.mult, OP.add))
            V(lambda e, pw=pw, po=po: e.tensor_tensor(
                ff.v(po, pw), ff.v(po, pw), mp.v(po, pw), OP.mult),
              waits=[('v', v_mp)])

        v_filt = pr.cnt['v']
        d_fout = pr.dma("fout", lambda e: e.dma_start(
            out=AP(Q, G1, [[CH, NPART], [1, CH]]), in_=ff.v()),
            waits=[('v', v_filt), d_z3, d_z4])

        xh.free()
        for bb in (diff[0], diff[1], wb[0], wb[1], wd[0], wd[1],
                   sden, rden, tqb, ff, mp):
            bb.free()

        # ============ phase 4: K median-fill stages (all on VE) =============
        mf = [ar.alloc("mf_a", MFW), ar.alloc("mf_b", MFW)]
        d_win = pr.dma("win", lambda e: e.dma_start(
            out=mf[0].v(), in_=AP(Q, G1 - HALO1, [[CH, NPART], [1, MFW]])),
            waits=[d_fout])

        hbuf = ar.alloc("hbuf", CH)
        g = [ar.alloc(f"g{i}", CH) for i in range(10)]
        NOFF = [-WP - 1, -WP, -WP + 1, -1, 1, WP - 1, WP, WP + 1]
        OFF1 = HALO1
        SORT8 = [(0, 1), (2, 3), (4, 5), (6, 7),
                 (0, 2), (1, 3), (4, 6), (5, 7),
                 (1, 2), (5, 6),
                 (0, 4), (1, 5), (2, 6), (3, 7),
                 (2, 4), (3, 5),
                 (1, 2), (3, 4), (5, 6)]

        for s in range(K):
            cur, nxt = mf[s % 2], mf[(s + 1) % 2]
            cc = cur.v(OFF1, CH)
            tv = [cur.v(OFF1 + o, CH) for o in NOFF]
            w_in = [d_win]

            # full Batcher sort-8 of the neighbor views (ascending, zeros =
            # invalid sink to the bottom).  Level 1 writes fresh buffers from
            # the views; later comparators use min->scratch, max->in-place.
            pos = [None] * 8
            for ci, (i, j) in enumerate(SORT8[:4]):
                TT(g[2 * ci].v(), tv[i], tv[j], OP.min, waits=w_in)
                TT(g[2 * ci + 1].v(), tv[i], tv[j], OP.max)
                pos[i], pos[j] = g[2 * ci], g[2 * ci + 1]
            scratch = g[8]
            for (i, j) in SORT8[4:]:
                TT(scratch.v(), pos[i].v(), pos[j].v(), OP.min)
                TT(pos[j].v(), pos[i].v(), pos[j].v(), OP.max)
                pos[i], scratch = scratch, pos[i]
            # pos == s0..s7 ascending

            # count-threshold masks from order-statistic positivity:
            # [cnt>=k] == [s_{8-k} > 0]
            m2 = mask8[:, 0 * CH:1 * CH]
            m4 = mask8[:, 1 * CH:2 * CH]
            m6 = mask8[:, 2 * CH:3 * CH]
            m8 = mask8[:, 3 * CH:4 * CH]
            V(lambda e, o=m2, i=pos[6]: e.tensor_scalar(
                o, i.v(), 0.0, None, OP.is_gt))
            V(lambda e, o=m4, i=pos[4]: e.tensor_scalar(
                o, i.v(), 0.0, None, OP.is_gt))
            V(lambda e, o=m6, i=pos[2]: e.tensor_scalar(
                o, i.v(), 0.0, None, OP.is_gt))
            V(lambda e, o=m8, i=pos[0]: e.tensor_scalar(
                o, i.v(), 0.0, None, OP.is_gt))
            s_h = V(lambda e, o=hbuf.v(), i=cc: e.tensor_scalar(
                o, i, 0.0, None, OP.is_le))

            # med = select by cnt
            med = g[9]
            V(lambda e, o=med.v(), i=pos[7]: e.tensor_copy(o, i.v()))
            V(lambda e, o=med.v(), m=m2, d=pos[6]: e.copy_predicated(o, m, d.v()))
            V(lambda e, o=med.v(), m=m4, d=pos[5]: e.copy_predicated(o, m, d.v()))
            V(lambda e, o=med.v(), m=m6, d=pos[4]: e.copy_predicated(o, m, d.v()))
            V(lambda e, o=med.v(), m=m8, d=pos[3]: e.copy_predicated(o, m, d.v()))

            # gate = (s7>0) * hole * padmask ; out = cc + med*gate
            gb = pos[0]   # reuse as scratch (s0 no longer needed)
            V(lambda e, o=gb.v(), a=pos[7], b=hbuf.v(): e.scalar_tensor_tensor(
                o, a.v(), 0.0, b, OP.is_gt, OP.mult))
            TT(gb.v(), gb.v(), pm.v(), OP.mult)
            V(lambda e, o=hbuf.v(), a=med.v(), b=gb.v(): e.scalar_tensor_tensor(
                o, a, 0.0, b, OP.bypass, OP.mult))
            v_stage = TT(nxt.v(OFF1, CH), hbuf.v(), cc, OP.add)

            d_out = pr.dma("mout", lambda e, nxt=nxt: e.dma_start(
                out=AP(Q, G1, [[CH, NPART], [1, CH]]), in_=nxt.v(OFF1, CH)),
                waits=[('v', v_stage)])
            d_win = pr.dma("win", lambda e, nxt=nxt: e.dma_start(
                out=nxt.v(), in_=AP(Q, G1 - HALO1, [[CH, NPART], [1, MFW]])),
                waits=[d_out])

        v_med_end = pr.cnt['v']
        s_med_end = pr.cnt['s']
        for bb in g:
            bb.free()
        hbuf.free()
        pm.free()

        # ================= phase 5: normals =================
        mf_fin = mf[K % 2]
        outd = mf[(K + 1) % 2]

        d0h = ar.alloc("d0h", MFW)
        ucb = ar.alloc("ucb", MFW)
        vrb = ar.alloc("vrb", MFW)
        wme = [('v', v_med_end), ('s', s_med_end)]
        d_d0h = pr.dma("d0h", lambda e: e.dma_start(
            out=d0h.v(), in_=AP(P, G4 - HALO1, [[CH, NPART], [1, MFW]])),
            waits=wme)
        d_uc = pr.dma("uc", lambda e: e.dma_start(
            out=ucb.v(), in_=AP(ucol, G1 - HALO1, [[CH, NPART], [1, MFW]])),
            waits=wme)
        d_vc = pr.dma("vc", lambda e: e.dma_start(
            out=vrb.v(), in_=AP(vrow, G1 - HALO1, [[CH, NPART], [1, MFW]])),
            waits=wme)

        # blend: outd = d0 + flag*(mf_fin - d0)  (full halo extent)
        V(lambda e: e.scalar_tensor_tensor(
            outd.v(), mf_fin.v(), 0.0, d0h.v(), OP.bypass, OP.subtract),
          waits=[d_win, d_d0h])
        V(lambda e: e.scalar_tensor_tensor(
            outd.v(), outd.v(), ap_flag, d0h.v(), OP.mult, OP.add))
        mf_fin.free()
        d0h.free()

        # valid mask, camera Z
        vv = ar.alloc("vv", MFW)
        V(lambda e: e.tensor_scalar(vv.v(), outd.v(), 6.0, None, OP.is_le))
        V(lambda e: e.scalar_tensor_tensor(
            vv.v(), outd.v(), 0.1, vv.v(), OP.is_ge, OP.mult))
        Z = ar.alloc("Z", MFW)
        V(lambda e: e.tensor_tensor(Z.v(), outd.v(), vv.v(), OP.mult))
        vv.free()
        outd.free()

        gz = ar.alloc("gz", CH)
        gz2 = ar.alloc("gz2", CH)
        TT(gz.v(), Z.v(OFF1 + 1, CH), Z.v(OFF1 - 1, CH), OP.min)
        TT(gz2.v(), Z.v(OFF1 + WP, CH), Z.v(OFF1 - WP, CH), OP.min)
        TT(gz.v(), gz.v(), gz2.v(), OP.min)
        TT(gz.v(), gz.v(), Z.v(OFF1, CH), OP.min)
        gz2.free()

        # A = (u-cx)/fx in-place on ucb; B = (v-cy)/fy; X = A*Z; Y = B*Z
        V(lambda e: e.tensor_scalar(
            ucb.v(), ucb.v(), ap_cx, ap_rfx, OP.subtract, OP.mult), waits=[d_uc])
        V(lambda e: e.tensor_scalar(
            vrb.v(), vrb.v(), ap_cy, ap_rfy, OP.subtract, OP.mult), waits=[d_vc])
        X = ar.alloc("X", MFW)
        Y = ar.alloc("Y", MFW)
        V(lambda e: e.tensor_tensor(X.v(), ucb.v(), Z.v(), OP.mult))
        V(lambda e: e.tensor_tensor(Y.v(), vrb.v(), Z.v(), OP.mult))
        ucb.free()
        vrb.free()

        # central differences (output extent)
        dxX = ar.alloc("dxX", CH); dyX = ar.alloc("dyX", CH)
        dxY = ar.alloc("dxY", CH); dyY = ar.alloc("dyY", CH)
        dxZ = ar.alloc("dxZ", CH); dyZ = ar.alloc("dyZ", CH)
        for (db, src) in ((dxX, X), (dxY, Y), (dxZ, Z)):
            TT(db.v(), src.v(OFF1 + 1, CH), src.v(OFF1 - 1, CH), OP.subtract)
        for (db, src) in ((dyX, X), (dyY, Y), (dyZ, Z)):
            TT(db.v(), src.v(OFF1 + WP, CH), src.v(OFF1 - WP, CH), OP.subtract)
        X.free(); Y.free(); Z.free()

        # cross product n = dy_vec x dx_vec
        m1 = ar.alloc("m1", CH); m2b = ar.alloc("m2b", CH)
        nx = ar.alloc("nx", CH); ny = ar.alloc("ny", CH); nz = ar.alloc("nz", CH)
        TT(m1.v(), dyY.v(), dxZ.v(), OP.mult)
        TT(m2b.v(), dyZ.v(), dxY.v(), OP.mult)
        TT(nx.v(), m1.v(), m2b.v(), OP.subtract)
        TT(m1.v(), dyZ.v(), dxX.v(), OP.mult)
        TT(m2b.v(), dyX.v(), dxZ.v(), OP.mult)
        TT(ny.v(), m1.v(), m2b.v(), OP.subtract)
        TT(m1.v(), dyX.v(), dxY.v(), OP.mult)
        TT(m2b.v(), dyY.v(), dxX.v(), OP.mult)
        v_nz = TT(nz.v(), m1.v(), m2b.v(), OP.subtract)
        for bb in (dxX, dyX, dxY, dyY, dxZ, dyZ, m1, m2b):
            bb.free()

        # nn2 = nx^2+ny^2+nz^2 (squares on ACT), nn = sqrt, rinv = 1/nn
        sq1 = ar.alloc("sq1", CH); sq2 = ar.alloc("sq2", CH); sq3 = ar.alloc("sq3", CH)
        pr.op('s', lambda e: e.activation(sq1.v(), nx.v(), AF.Square),
              waits=[('v', v_nz)])
        pr.op('s', lambda e: e.activation(sq2.v(), ny.v(), AF.Square))
        s_q3 = pr.op('s', lambda e: e.activation(sq3.v(), nz.v(), AF.Square))
        nn2 = sq1
        TT(nn2.v(), sq1.v(), sq2.v(), OP.add, waits=[('s', s_q3)])
        TT(nn2.v(), nn2.v(), sq3.v(), OP.add)
        qb = sq2
        V(lambda e: e.tensor_scalar(qb.v(), nn2.v(), 1e-16, None, OP.is_gt))
        v_nn2 = V(lambda e: e.tensor_scalar(nn2.v(), nn2.v(), 1e-30, None, OP.max))
        nn = sq3
        s_nn = pr.op('s', lambda e: e.activation(nn.v(), nn2.v(), AF.Sqrt),
                     waits=[('v', v_nn2)])
        rinv = nn2
        V(lambda e: e.reciprocal(rinv.v(), nn.v()), waits=[('s', s_nn)])

        # gate = (zmin>0)*q ; rg = rinv*gate ; n_out = n * rg
        V(lambda e: e.scalar_tensor_tensor(
            gz.v(), gz.v(), 0.0, qb.v(), OP.is_gt, OP.mult))
        TT(gz.v(), gz.v(), rinv.v(), OP.mult)
        v_ox = TT(nx.v(), nx.v(), gz.v(), OP.mult)
        v_oy = TT(ny.v(), ny.v(), gz.v(), OP.mult)
        v_oz = TT(nz.v(), nz.v(), gz.v(), OP.mult)

        # ================= phase 6: outputs =================
        fouts = []
        for i, (buf, vdone) in enumerate(((nx, v_ox), (ny, v_oy), (nz, v_oz))):
            d_o = pr.dma(f"o{i}", lambda e, i=i, buf=buf: e.dma_start(
                out=AP(O, i * NPIX, [[CH, NPART], [1, CH]]), in_=buf.v()),
                waits=[('v', vdone)])
            fouts.append(pr.dma(f"f{i}", lambda e, i=i: e.dma_start(
                out=AP(out, i * H * W, [[W, H], [1, W]]),
                in_=AP(O, i * NPIX + 4, [[WP, H], [1, W]])),
                waits=[d_o]))
        for bb in (nx, ny, nz, sq1, sq2, sq3, gz):
            bb.free()
        return fouts

    fouts = []
    for r in range(R):
        fouts = emit_iter(fouts)

    # ================= emit =================
    import contextlib
    with contextlib.ExitStack() as stack:
        block = stack.enter_context(nc.Block())
        sems = {k: stack.enter_context(nc.semaphore(f"sem_{k}")) for k in 'vsyt'}
        dsems = {name: stack.enter_context(nc.semaphore(f"dma_{name}"))
                 for name in pr.dma_count}

        def replay(key):
            def run(eng):
                own = sems[key]
                first = True
                for waits, fn, inc in pr.items[key]:
                    for wt in waits:
                        if wt[0] == 'D':
                            eng.wait_ge(dsems[wt[1]], 16 * wt[2])
                        else:
                            eng.wait_ge(sems[wt[0]], wt[1])
                    if DRAIN_VS and key in 'vs' and not first:
                        eng.drain()
                    first = False
                    inst = fn(eng)
                    if isinstance(inc, tuple):
                        inst.then_inc(dsems[inc[1]], 16)
                    else:
                        inst.then_inc(own, inc)
            return run

        block.vector(replay('v'))
        block.scalar(replay('s'))
        block.sync(replay('y'))
        block.tensor(replay('t'))

    return nc


# ---------------- host side ----------------

_PROG_CACHE = {}
LAST_RESULTS = None


def _fill_stages_needed(holes):
    """Exact mask-only simulation of the reference's (up to 7) median fills."""
    z = holes.copy()
    Kmax = 0
    for step in range(7):
        nzp = np.pad(~z, ((0, 0), (1, 1), (1, 1)))
        any_nb = np.zeros_like(z)
        for dy in (0, 1, 2):
            for dx in (0, 1, 2):
                if dy == 1 and dx == 1:
                    continue
                any_nb |= nzp[:, dy:dy + z.shape[1], dx:dx + z.shape[2]]
        fill = z & any_nb
        if not fill.any():
            break
        z &= ~fill
        Kmax = step + 1
    return Kmax


def _const_maps():
    gidx = np.arange(NPIX, dtype=np.int64)
    u = (gidx % WP - 4).astype(np.float32)
    v = (gidx // WP).astype(np.float32)
    uc = np.zeros(QSZ, np.float32)
    vr = np.zeros(QSZ, np.float32)
    uc[G1:G1 + NPIX] = u
    vr[G1:G1 + NPIX] = v
    return uc, vr


def _ident_map():
    """NID scaled identity tiles: s_j = ws(r2_j) * sqrt(pi)/2."""
    m = np.zeros((NPART, NID * NPART), np.float32)
    eye = np.eye(NPART, dtype=np.float32)
    for j, r2 in enumerate(R2S):
        s = np.float32(math.exp(-r2 / 8.0) * math.sqrt(math.pi) / 2.0)
        m[:, j * NPART:(j + 1) * NPART] = eye * s
    return m


def _in_maps(d0, intr, flags):
    uc, vr = _const_maps()
    idm = _ident_map()
    maps = []
    for b in range(B):
        i8 = np.zeros(8, np.float32)
        i8[0:4] = intr[b]
        i8[4] = flags[b]
        maps.append({
            "depth": np.ascontiguousarray(d0[b]),
            "intr8": np.tile(i8, (NPART, 1)),
            "ucol": uc,
            "vrow": vr,
            "ident": idm,
        })
    return maps


def kernel(depth, intrinsic_params, _trace=False):
    global LAST_RESULTS
    from concourse.bass_utils import run_bass_kernel_spmd

    depth = np.asarray(depth, np.float32)
    intr = np.asarray(intrinsic_params, np.float32)
    d0 = depth[:, 0]                       # [B,H,W]
    holes = d0 == 0.0
    flags = holes.any(axis=(1, 2)).astype(np.float32)
    K = _fill_stages_needed(holes) if flags.any() else 0

    if K not in _PROG_CACHE:
        _PROG_CACHE[K] = build_program(K)
    nc = _PROG_CACHE[K]

    in_maps = _in_maps(d0, intr, flags)
    res = run_bass_kernel_spmd(nc, in_maps, core_ids=list(range(B)), trace=_trace)
    LAST_RESULTS = res
    return np.stack([r["out"] for r in res.results]).astype(np.float32)
